# revision 35
# baseline (speedup 1.0000x reference)
"""Trainium2 Bass kernel: batched RK4 integration of a tiny 2-4-1 LeakyReLU MLP ODE.

Math (per batch element, 99 RK4 steps, dt=1):
  dyn(s) = b2 + sum_j w_j * lrelu(a_j*s + c_j),  a=W1[0,:], c_j=W1[1,j]*u+b1_j

Device formulation — j lives on the PARTITION dim (128 = 4j x 32 rows;
16384 elems/core as [32 rows x 512 cols], elem e = r*512 + c):
  y_j = s + d_j,   d_j = c_j / a_j
  State V[j*32+r, c] = sign(a_j) * y_j(e)     (fp32, exact)
  lrelu:  w_j*lrelu(a_j*y) = w_j*|a_j| * max(V_j, 0.01*V_j)
          -> U' = max(V, 0.01 V): one ACT Prelu (f32r output)
  stage inputs land directly in PSUM via PE matmuls:
      Z_i = t_i*sign(a)*k~ + Vr       (W(t_i)@U + I@Vr; Vr = f32r copy of V)
      k~ = sum_j (w_j|a_j|) U'_j      (per-j signs/factors in the weights)
      the t_i*b2*sign(a) constant folds into the Prelu's per-partition bias
  step:   KK = sign(a)*(k1+2k2+2k3+k4+6 b2)  (4 coef matmuls + ones matmul)
          V' = KK/6 + V   (DVE stt — EXACT fp32 state; Vr rounding only
                           perturbs the k-evaluations, ~1e-4 relative)
          Vr' = KK/6 + V  (same inputs, f32r output, for next step's I@Vr)
          s_t = sign(a_g)*V'_g - d_g, rows g = t%4  (single Pool tensor_tensor)
Two column groups (256 cols each — float32r needs >=256 for 1 cycle/row).
The wall-clock is chain-bound: 4 serial (Prelu -> matmul) rounds per step.
"""

import sys
import numpy as np

sys.path.insert(0, "/opt/trn_rl_repo")

B = 131072
T = 100
P = 128
NCORES = 8
PER = B // NCORES          # 16384 elements per core
RROWS = 32                 # element rows per j-block
COLS = PER // RROWS        # 512 element columns
NBLK = T // 4              # TRJ col-blocks (4 steps each)

CONFIG = {
    "G": 2,
    "chunk_blks": 4,       # col-blocks per output DMA
    "l1_dve": 0,           # cols of stage-1 lrelu on DVE (rest on ACT)
    "vr_pool": False,      # Vr' copy on Pool (tensor_scalar) instead of DVE stt
    "u12_pool": True,      # merge U1+U2 (one W2 matmul for both stages)
    "u12_dve": True,       # ...with the add on DVE, not Pool
    "strack_pool": False,  # trajectory extraction on Pool (else DVE)
    "l1_chain": True,      # stage-1 lrelu as DVE stt chained right after VP
    "vr_act": True,        # Vr f32r copy on ACT (DVE chain: VP, U0, strack)
    "w_first": False,      # stage-2 Z-bank order [I@Vr, W@U0] (Vr early on ACT)
}

# init tensor column layout
C_V0 = 0
C_ND = COLS                         # negD
C_W = 2 * COLS                      # weight pack: Wh, Wf, W2, WI, Wb
NW = 5
C_B05 = C_W + NW * P
C_B10 = C_B05 + 1
NINIT = C_B10 + 1


def _numpy_fallback(x, u, W1, b1, W2, b2):
    s = x[:, 0].astype(np.float32)
    uu = u[:, 0].astype(np.float32)
    traj = [s.copy()]
    for _ in range(T - 1):
        def dyn(ss):
            z = np.stack([ss, uu], axis=-1)
            h = z @ W1 + b1
            h = np.where(h >= 0, h, np.float32(0.01) * h)
            return (h @ W2)[:, 0] + b2[0]
        k1 = dyn(s)
        k2 = dyn(s + np.float32(0.5) * k1)
        k3 = dyn(s + np.float32(0.5) * k2)
        k4 = dyn(s + k3)
        s = s + np.float32(1 / 6) * (k1 + 2 * k2 + 2 * k3 + k4)
        traj.append(s.copy())
    out = np.stack(traj, axis=1).astype(np.float32)
    return out[:, :, None]


def _build_program(sg, cfg=None):
    """sg: tuple of 4 signs of a_j (+1/-1) — baked into strack ops."""
    from concourse import bacc, tile, mybir
    from concourse.bass_types import AP

    cfg = dict(CONFIG, **(cfg or {}))
    G = cfg["G"]
    GW = COLS // G
    CB = cfg["chunk_blks"]

    AF = mybir.ActivationFunctionType
    ALU = mybir.AluOpType
    f32 = mybir.dt.float32
    f32r = mybir.dt.float32r
    nc = bacc.Bacc("TRN2", target_bir_lowering=False, debug=False)

    init = nc.dram_tensor("init", [P, NINIT], f32, kind="ExternalInput")
    out = nc.dram_tensor("out", [T, PER], f32, kind="ExternalOutput")

    with tile.TileContext(nc) as tc:
        with tc.tile_pool(name="main", bufs=1) as pool, \
             tc.tile_pool(name="ps", bufs=1, space="PSUM") as pp:
            INIT = pool.tile([P, NINIT], f32)
            TRJ = pool.tile([P, NBLK * COLS], f32)
            WR = pool.tile([P, NW * P], f32r)
            ONESF = pool.tile([P, GW], f32)
            ONESR = pool.tile([P, GW], f32r)
            V = [[pool.tile([P, GW], f32, name=f"V_{g}_{i}") for i in range(2)]
                 for g in range(G)]
            VRT = [[pool.tile([P, GW], f32r, name=f"VR_{g}_{i}") for i in range(2)]
                   for g in range(G)]
            VRB = [[pool.tile([P, GW], f32r, name=f"VRB_{g}_{i}")
                    for i in range(2)] for g in range(G)]
            U = [[pool.tile([P, GW], f32r, name=f"U_{g}_{i}") for i in range(4)]
                 for g in range(G)]
            U12 = [pool.tile([P, GW], f32r, name=f"U12_{g}") for g in range(G)]
            # psum: one full bank per matmul accumulation target (start=True
            # resets at bank granularity — never share a bank between groups)
            ZBT = [[pp.tile([P, GW], f32, name=f"ZB_{g}_{i}") for i in range(3)]
                   for g in range(G)]
            KKT = [pp.tile([P, GW], f32, name=f"KK_{g}") for g in range(G)]
            ZB = [[ZBT[g][i][:] for i in range(3)] for g in range(G)]
            KK = [KKT[g][:] for g in range(G)]

            nc.sync.dma_start(INIT[:], init.ap())
            nc.scalar.activation(WR[:], INIT[:, C_W:C_W + NW * P],
                                 AF.Copy, bias=0.0, scale=1.0)
            Wh = WR[:, 0:P]
            Wf = WR[:, P:2 * P]
            W2 = WR[:, 2 * P:3 * P]
            WI = WR[:, 3 * P:4 * P]
            Wb = WR[:, 4 * P:5 * P]
            kk6 = cfg.get("kk6", False)
            if not kk6:
                nc.vector.memset(ONESF[:], 1.0)
                nc.scalar.activation(ONESR[:], ONESF[:], AF.Copy, bias=0.0,
                                     scale=1.0)
            b05 = INIT[:, C_B05:C_B05 + 1]
            b10 = INIT[:, C_B10:C_B10 + 1]

            for g in range(G):
                v0 = INIT[:, C_V0 + g * GW:C_V0 + (g + 1) * GW]
                nc.scalar.activation(V[g][0][:], v0, AF.Copy, bias=0.0, scale=1.0)
                nc.scalar.activation(VRT[g][0][:], v0, AF.Copy, bias=0.0,
                                     scale=1.0)
                if cfg.get("l4_dve", False):
                    nc.scalar.activation(VRB[g][0][:], v0, AF.Identity,
                                         bias=INIT[:, C_B10:C_B10 + 1],
                                         scale=1.0)

            def strack(t, g, vrows):
                gg = t % 4
                p0, p1 = gg * RROWS, (gg + 1) * RROWS
                c0 = (t // 4) * COLS + g * GW
                nd = INIT[p0:p1, C_ND + g * GW:C_ND + (g + 1) * GW]
                eng = nc.gpsimd if cfg["strack_pool"] else nc.vector
                dst = TRJ[p0:p1, c0:c0 + GW]
                if sg[gg] > 0:
                    eng.tensor_tensor(dst, vrows(p0, p1), nd, ALU.add)
                else:
                    eng.tensor_tensor(dst, nd, vrows(p0, p1), ALU.subtract)

            for g in range(G):
                strack(0, g, lambda p0, p1, g=g:
                       INIT[p0:p1, C_V0 + g * GW:C_V0 + (g + 1) * GW])

            def dma_chunk(b):
                b0 = (b // CB) * CB
                nb = b - b0 + 1
                trj_ap = TRJ[:]
                src = AP(trj_ap.tensor, trj_ap.offset + b0 * COLS,
                         [trj_ap.ap[0], [COLS, nb], [1, COLS]])
                out_ap = out.ap()
                dst = AP(out_ap.tensor, out_ap.offset + b0 * 4 * PER,
                         [[PER, 4], [COLS, RROWS], [4 * PER, nb], [1, COLS]])
                nc.sync.dma_start(dst, src)

            VPQ = []
            l1d = cfg["l1_dve"]
            l1a = GW - l1d

            l1_chain = cfg.get("l1_chain", False)

            def emit_l1(g, vc):
                if l1_chain:
                    nc.vector.scalar_tensor_tensor(
                        U[g][0][:], vc[:], 0.01, vc[:], ALU.mult, ALU.max)
                    return
                if l1a:
                    nc.scalar.activation(U[g][0][:, 0:l1a], vc[:, 0:l1a],
                                         AF.Prelu, bias=0.0, scale=1.0,
                                         alpha=0.01)
                if l1d:
                    nc.vector.scalar_tensor_tensor(
                        U[g][0][:, l1a:GW], vc[:, l1a:GW], 0.01,
                        vc[:, l1a:GW], ALU.mult, ALU.max)

            for t in range(1, T):
                cur, nxt = (t - 1) % 2, t % 2
                if t == 1 or not l1_chain:
                    # step-1 (or non-chained mode): stage-1 lrelu from V[cur]
                    for g in range(G):
                        emit_l1(g, V[g][cur])
                u12 = cfg["u12_pool"]
                if not kk6:
                    for g in range(G):
                        # KK group opens with the constant ones term
                        nc.tensor.matmul(KK[g][:], Wb, ONESR[:],
                                         start=True, stop=False)
                l4d = cfg.get("l4_dve", False)
                for si, (wz, bias) in enumerate(
                        [(Wh, b05), (Wh, b05), (Wf, b10)]):
                    for g in range(G):
                        vr_ap = (VRB[g][cur] if (l4d and si == 2)
                                 else VRT[g][cur])[:]
                        if si == 0 and l1_chain and cfg.get("w_first", True):
                            # U0 (chained stt) lands before Vr: reduce first
                            nc.tensor.matmul(ZB[g][si][:], wz, U[g][si][:],
                                             start=True, stop=False)
                            nc.tensor.matmul(ZB[g][si][:], WI, vr_ap,
                                             start=False, stop=True)
                        else:
                            # Z-bank: I@Vr first (ready early), then U-reduce
                            nc.tensor.matmul(ZB[g][si][:], WI, vr_ap,
                                             start=True, stop=False)
                            nc.tensor.matmul(ZB[g][si][:], wz, U[g][si][:],
                                             start=False, stop=True)
                    for g in range(G):
                        # off-chain KK accumulation after both groups' Z-mms
                        if si == 0:
                            nc.tensor.matmul(KK[g][:], Wb if kk6 else Wf,
                                             U[g][0][:],
                                             start=kk6, stop=False)
                        elif not u12:
                            nc.tensor.matmul(KK[g][:], W2, U[g][si][:],
                                             start=False, stop=False)
                    for g in range(G):
                        if l4d and si == 2:
                            nc.vector.scalar_tensor_tensor(
                                U[g][si + 1][:], ZB[g][si][:], 0.01,
                                ZB[g][si][:], ALU.mult, ALU.max)
                        else:
                            nc.scalar.activation(U[g][si + 1][:], ZB[g][si][:],
                                                 AF.Prelu, bias=bias, scale=1.0,
                                                 alpha=0.01)
                    if si == 1 and u12:
                        ueng = nc.vector if cfg.get("u12_dve", False) \
                            else nc.gpsimd
                        for g in range(G):
                            ueng.tensor_tensor(U12[g][:], U[g][1][:],
                                               U[g][2][:], ALU.add)
                for g in range(G):
                    if u12:
                        nc.tensor.matmul(KK[g][:], W2, U12[g][:],
                                         start=False, stop=False)
                    nc.tensor.matmul(KK[g][:], Wb if kk6 else Wf,
                                     U[g][3][:], start=False, stop=True)
                for g in range(G):
                    def emit_vp(g=g, cur=cur, nxt=nxt):
                        if kk6:
                            nc.vector.scalar_tensor_tensor(
                                V[g][nxt][:], KK[g][:], b10,
                                V[g][cur][:], ALU.add, ALU.add)
                        else:
                            nc.vector.scalar_tensor_tensor(
                                V[g][nxt][:], KK[g][:], float(1.0 / 6.0),
                                V[g][cur][:], ALU.mult, ALU.add)

                    def emit_vr(g=g, cur=cur, nxt=nxt):
                        if l1_chain and cfg.get("vr_act", False):
                            nc.scalar.activation(VRT[g][nxt][:], V[g][nxt][:],
                                                 AF.Copy, bias=0.0, scale=1.0)
                        elif l1_chain and cfg["vr_pool"]:
                            nc.gpsimd.tensor_scalar(VRT[g][nxt][:],
                                                    V[g][nxt][:],
                                                    1.0, None, ALU.mult)
                        elif l1_chain:
                            # read V' so this stays behind the chained L1 stt
                            nc.vector.tensor_scalar(VRT[g][nxt][:],
                                                    V[g][nxt][:],
                                                    1.0, None, ALU.mult)
                        elif cfg["vr_pool"]:
                            nc.gpsimd.tensor_scalar(VRT[g][nxt][:],
                                                    V[g][nxt][:],
                                                    1.0, None, ALU.mult)
                        elif cfg.get("vr_act", False):
                            nc.scalar.activation(VRT[g][nxt][:], V[g][nxt][:],
                                                 AF.Copy, bias=0.0, scale=1.0)
                        else:
                            nc.vector.scalar_tensor_tensor(
                                VRT[g][nxt][:], KK[g][:], float(1.0 / 6.0),
                                V[g][cur][:], ALU.mult, ALU.add)
                    if l1_chain:
                        emit_vp()
                        if t < T - 1:
                            emit_l1(g, V[g][nxt])
                        emit_vr()
                        if cfg.get("l4_dve", False) and t < T - 1:
                            if cfg.get("vrb_act", False):
                                nc.scalar.activation(
                                    VRB[g][nxt][:], V[g][nxt][:], AF.Identity,
                                    bias=INIT[:, C_B10:C_B10 + 1], scale=1.0)
                            else:
                                nc.gpsimd.tensor_scalar(
                                    VRB[g][nxt][:], V[g][nxt][:],
                                    INIT[:, C_B10:C_B10 + 1], None, ALU.add)
                    elif cfg.get("vr_first", False):
                        emit_vr()
                        emit_vp()
                    elif cfg.get("vp_grouped", False):
                        VPQ.append(emit_vr)
                        emit_vp()
                    else:
                        emit_vp()
                        emit_vr()
                if cfg.get("vp_grouped", False):
                    for f in VPQ:
                        f()
                    VPQ.clear()
                for g in range(G):
                    if cfg.get("strack_split", False):
                        sp = (g == 1)
                    elif cfg.get("strack_alt", False):
                        sp = (t % 2 == 0)
                    else:
                        sp = cfg["strack_pool"]
                    eng_save = cfg["strack_pool"]
                    cfg["strack_pool"] = sp
                    strack(t, g, lambda p0, p1, g=g, nxt=nxt:
                           V[g][nxt][p0:p1, :])
                    cfg["strack_pool"] = eng_save
                if t % 4 == 3:
                    b = t // 4
                    if (b + 1) % CB == 0 or b == NBLK - 1:
                        dma_chunk(b)
    if not nc.is_finalized():
        nc.finalize()
    return nc


_PROGRAM_CACHE = {}


def kernel(x, u, W1, b1, W2, b2):
    x = np.asarray(x, dtype=np.float32)
    u = np.asarray(u, dtype=np.float32)
    W1 = np.asarray(W1, dtype=np.float32)
    b1 = np.asarray(b1, dtype=np.float32)
    W2 = np.asarray(W2, dtype=np.float32)
    b2 = np.asarray(b2, dtype=np.float32)

    a = W1[0, :].astype(np.float64)
    w = W2[:, 0].astype(np.float64)
    if x.shape != (B, 1) or np.any(np.abs(a) < 1e-6):
        return _numpy_fallback(x, u, W1, b1, W2, b2)

    from concourse import bass_utils

    sg = tuple(1 if v > 0 else -1 for v in a)
    nc = _PROGRAM_CACHE.get(sg)
    if nc is None:
        nc = _build_program(sg)
        _PROGRAM_CACHE[sg] = nc

    b2f = float(b2[0])
    sga = np.array(sg, dtype=np.float64)
    wa = w * np.abs(a)                       # w_j * |a_j|

    eye = np.eye(RROWS, dtype=np.float64)
    Wf = np.zeros((P, P))
    for j in range(4):
        for jp in range(4):
            Wf[j * RROWS:(j + 1) * RROWS, jp * RROWS:(jp + 1) * RROWS] = \
                eye * (wa[j] * sga[jp])
    Wh = 0.5 * Wf
    WI = np.eye(P)
    if CONFIG.get("kk6", False):
        W2m = Wf / 3.0
        Wb = Wf / 6.0
    else:
        W2m = 2.0 * Wf
        Wb = np.tile(np.repeat(6.0 * b2f * sga / P, RROWS)[None, :], (P, 1))

    c = u[:, 0].astype(np.float64)[:, None] * W1[1, :].astype(np.float64)[None, :] \
        + b1.astype(np.float64)[None, :]      # [B,4]
    d = c / a[None, :]

    mj_sign = np.repeat(sga, RROWS)           # [128]

    in_maps = []
    for core in range(NCORES):
        sl = slice(core * PER, (core + 1) * PER)
        xe = x[sl, 0].astype(np.float64)
        de = d[sl]
        v0 = (xe[:, None] + de) * sga[None, :]                  # [16384, 4]
        V0 = v0.reshape(RROWS, COLS, 4).transpose(2, 0, 1).reshape(P, COLS)
        ND = (-de).reshape(RROWS, COLS, 4).transpose(2, 0, 1).reshape(P, COLS)
        ini = np.zeros((P, NINIT), dtype=np.float32)
        ini[:, C_V0:C_V0 + COLS] = V0.astype(np.float32)
        ini[:, C_ND:C_ND + COLS] = ND.astype(np.float32)
        for k, Wm in enumerate([Wh, Wf, W2m, WI, Wb]):
            ini[:, C_W + k * P:C_W + (k + 1) * P] = Wm.astype(np.float32)
        ini[:, C_B05] = (0.5 * b2f) * mj_sign
        ini[:, C_B10] = b2f * mj_sign
        in_maps.append({"init": ini})

    res = bass_utils.run_bass_kernel_spmd(nc, in_maps, list(range(NCORES)))

    outf = np.empty((B, T), dtype=np.float32)
    for core in range(NCORES):
        dev = np.asarray(res.results[core]["out"]).reshape(T, PER)
        outf[core * PER:(core + 1) * PER, :] = dev.T
    return outf[:, :, None]
